# revision 40
# baseline (speedup 1.0000x reference)
"""DCL contrastive loss kernel for Trainium2 (8 NeuronCores, Bass/Tile).

Problem: u, v [8192, 256] f32.
  sim_uv = cos_sim(u, v) / T ; sim_uu = cos_sim(u, u) / T   (T = 0.07)
  loss = mean_i( -sim_uv[i,i] + logsumexp_j(off-diag of [sim_uv | sim_uu] row i) )

Strategy (data-parallel rows, per the sharding hint), v3:
  Phase 1 (SPMD, 8 cores): each core normalizes its 1024-row shard of u and v
    (ACT Square+accum row norms, DVE reciprocal), emits FP8 (e4m3) unit rows
    TRANSPOSED into the DoubleRow matmul layout [128, 2, 1024]. Input rows
    map r -> (partition r//8, line r%8) so every DMA line is >=4KB
    contiguous; the resulting column permutation (col j*128+p <-> shard row
    8p+j) is undone on the host (COLPERM). All DMAs ride the two HWDGE
    rings (sync + scalar) -- gpsimd SWDGE descriptor emission measured
    ~60us of pure overhead here.
  Host: concatenate the 8 shards column-ROLLED per core (own rows first) so
    every core's uu diagonal block sits at a static column offset.
  Phase 2 (SPMD, 8 cores): each core computes its [1024 x 8192] slab of both
    similarity matrices with fp8 DoubleRow matmuls (0.5 cyc/col, full K=256
    per instruction) into a 2-deep [128, 2048] PSUM ring, drained by TWO
    engines in parallel (HW-calibrated 44:20 tile split):
      - ACT "A" tiles: hardware exp activation, fused row-sum accum_out
        (1.66us/tile).
      - DVE "G" tiles: bf16 Schraudolph exp (i16 = A16*C*ps + B16, bitcast
        bf16 ~ exp(C(ps-1))), then a 2-level pairwise bf16 tree-fold and a
        reduce -- ALL on DVE (3.6us/tile), so nothing in DVE's in-order
        stream ever waits on another engine (cross-engine chains through
        GPSIMD measured 30-50us of head-of-line stalls and were removed).
    The uu diagonal block is masked by an extra PE matmul accumulating
    -6*I into PSUM before exp (contribution -> e^-86, i.e. 0); the uv
    diagonal is left in and corrected exactly on the host.
    Output: negsum [128, 8] f32 per core (row sums of exp((s-1)C) over both
    matrices, uu diag excluded), in urT column order.
  Host: loss_i = ln(negsum_i - diag corrections) + C - C*d_i with d_i the
    exact f64 diag cos(u_i, v_i) taken in column order; mean in f64.

The `loop_n` build parameter wraps the phase body in an on-device For_i loop
(used only for benchmarking; launch overhead cancels via the loop delta).
"""

import os
import sys

for _p in ("/opt/trn_rl_repo",):
    if _p not in sys.path:
        sys.path.insert(0, _p)

os.environ.setdefault("BASS_NEVER_TRACE", "1")

from contextlib import ExitStack

import numpy as np

import concourse.bass as bass
import concourse.tile as tile
from concourse import bacc, mybir
from concourse.bass_utils import run_bass_kernel_spmd

NCORES = 8
B, D = 8192, 256
SH = B // NCORES      # 1024 rows per core
PB = 128              # partition block
MB = SH // PB         # 8 row blocks per core
KD = D // PB          # 2 contraction halves
TEMP = 0.07
C = float(1.0 / TEMP)
SEG = 2048            # columns per DMA input tile
NSEG = B // SEG       # 4 input segments per matrix
GROUP = 2048          # columns per exp/accumulate group (4 PSUM banks)
NG = B // GROUP       # 4 groups per matrix
CHUNK = 512           # matmul chunk (1 PSUM bank)
NQ = GROUP // CHUNK
MASK = 6.0            # uu diag mask: s -> s - MASK => exp arg ~ -C*(1+MASK)

# Schraudolph: i32 = A_S*x + B_S, bitcast -> ~e^x  (zero-mean-error shift)
_LOG2E = 1.4426950408889634
A_S = float((1 << 23) * _LOG2E)
B_S = float(127.0 * (1 << 23) - 0.056446 * (1 << 23))
# bf16 variant: i16 = A16*x + B16, bitcast bf16 -> ~e^x
A16 = float((1 << 7) * _LOG2E)
B16 = float(127.0 * (1 << 7) - 0.056446 * (1 << 7))

F32 = mybir.dt.float32
BF16 = mybir.dt.bfloat16
FP8 = mybir.dt.float8e4
I32 = mybir.dt.int32
I16 = mybir.dt.int16
ALU = mybir.AluOpType
ACT = mybir.ActivationFunctionType
DR = mybir.MatmulPerfMode.DoubleRow

_PROGRAMS = {}

# --- static two-engine drain schedule for the 64 exp tiles ---------------


def _plan_exp_schedule():
    """Static balanced assignment of the 128 (m, slot) [128,1024] exp tiles:
    'A' = ACT exp with fused row-sum accumulator, 'G' = DVE Schraudolph feed
    consumed by a gpsimd per-m accumulator chain (sim-measured rates: ACT
    ~1164 ns, DVE feed ~1237 ns, GP add ~985 ns, per-m acc reduce ~1124 ns
    -> ACT/DVE/GP land near 82 us). Interleaved for the 4-deep PSUM ring."""
    # HW-calibrated rates: ACT exp 425 ns, DVE feed 647 ns, DVE bitcast
    # reduce ~2.1 us, gpsimd add ~2.6 us (serial chain). ACT-heavy is right:
    # 13 ACT + 3 gpsimd-chain tiles per m balances ACT/DVE/GP near 43 us.
    # Mixed G/A: with no SWDGE DMA emission left on the Pool queue (all
    # DMAs moved to the two HWDGE rings), the gpsimd chain is viable again.
    # Sim rates: ACT exp 2.07us, DVE feed 2.13us, GP add 4.16us per
    # [128,2048] tile. Alternating 3G/4G rows balance all three engines
    # near 75-83us total (ACT 74.5, DVE 76.6 w/ folds, GP 83).
    # Slot 7 (the masked uu-diag group) stays "A": its diag exp arg is
    # ~-86, within ~1.5 units of the Schraudolph i32 sign wrap.
    # G tiles are fully self-contained on DVE (feed 2.26us from PSUM +
    # bf16 tree-fold 0.92us + reduce 0.59us = 3.77us), so no cross-engine
    # latency ever enters DVE's in-order stream. ACT tile = 2.08us.
    # 6:2 mix of 3G/2G rows balances ACT 87us / DVE 85us.
    # G spread 0/3/6 so the 2-deep PSUM ring always has an ACT tile and a
    # DVE tile adjacent (no same-reader serialization)
    # HW-calibrated (microbench 2026-08-10): ACT exp 1.657us/tile, DVE
    # G-chain 3.60us/tile, PE stream 1.39us/tile, mixed pattern paces at
    # ~1.67us/tile. Balance 44A:20G -> ACT 72.9us ~= DVE 72us.
    pat3 = ["G", "A", "A", "G", "A", "A", "G", "A"]
    pat2 = ["G", "A", "A", "A", "G", "A", "A", "A"]
    plan = {}
    for m in range(MB):
        pat = pat2 if m % 2 == 1 else pat3
        for slot in range(2 * NG):
            plan[(m, slot)] = pat[slot]
    return plan


def _build_phase1(loop_n=0):
    """Normalize shard rows, emit fp8 transposed DoubleRow layout:
    us, vs [1024, 256] f32 -> unT, vnT [128, 2, 1024] fp8."""
    nc = bacc.Bacc("TRN2", target_bir_lowering=False, debug=False)
    us = nc.dram_tensor("us", [SH, D], F32, kind="ExternalInput")
    vs = nc.dram_tensor("vs", [SH, D], F32, kind="ExternalInput")
    id8 = nc.dram_tensor("id8", [PB, PB], BF16, kind="ExternalInput")
    unT = nc.dram_tensor("unT", [PB, KD, SH], FP8, kind="ExternalOutput")
    vnT = nc.dram_tensor("vnT", [PB, KD, SH], FP8, kind="ExternalOutput")

    with tile.TileContext(nc) as tc, ExitStack() as ctx:
        pool = ctx.enter_context(tc.tile_pool(name="main", bufs=2))
        sp = ctx.enter_context(tc.tile_pool(name="small", bufs=4))
        consts = ctx.enter_context(tc.tile_pool(name="consts", bufs=1))
        psum = ctx.enter_context(
            tc.tile_pool(name="psum", bufs=2, space=bass.MemorySpace.PSUM)
        )
        idt = consts.tile([PB, PB], BF16, tag="idt")
        nc.sync.dma_start(idt[:], id8[:])

        def body():
            for mi, (src, dst) in enumerate(((us, unT), (vs, vnT))):
                # both queues are HWDGE rings (SP + ACT): gpsimd SWDGE
                # descriptor emission (~5-25us/DMA) was phase1's bottleneck
                qin = nc.sync if mi == 0 else nc.scalar
                ss = sp.tile([PB, MB], F32, tag=f"ss{mi}", name=f"ss{mi}")
                H = MB // 2
                xh = []
                # row r -> (p, t) = (r // MB, r % MB): each partition line
                # is 4KB contiguous (4x1KB rows) instead of 4 scattered 1KB
                # rows -- fewer, larger DMA descriptors. The resulting
                # column permutation (urT col j*128+p <-> shard row 8p+j)
                # is undone on the host (COLPERM).
                xsrc = src.rearrange("(p t) d -> p t d", p=PB)
                for h in range(2):
                    x = pool.tile([PB, H, D], F32, tag=f"x{mi}{h}",
                                  name=f"x{mi}{h}")
                    qin.dma_start(x[:], xsrc[:, h * H:(h + 1) * H, :])
                    xh.append(x)
                    sq = pool.tile([PB, H, D], F32, tag=f"sq{mi}{h}",
                                   name=f"sq{mi}{h}")
                    for t in range(H):
                        nc.scalar.activation(
                            sq[:, t, :], x[:, t, :], ACT.Square,
                            accum_out=ss[:, h * H + t:h * H + t + 1])
                # r = 1/sqrt(ss); fp8 output tolerates the raw reciprocal
                # (ACT.Rsqrt is rejected by bass for accuracy reasons)
                nrm = sp.tile([PB, MB], F32, tag=f"nrm{mi}")
                nc.scalar.activation(nrm[:], ss[:], ACT.Sqrt)
                rn = sp.tile([PB, MB], F32, tag=f"rn{mi}")
                nc.vector.reciprocal(rn[:], nrm[:])
                # y = x * r -> bf16, one fused broadcast multiply per half
                y = pool.tile([PB, MB, D], BF16, tag=f"y{mi}")
                for h in range(2):
                    rbc = rn[:, h * H:(h + 1) * H].unsqueeze(-1).broadcast_to(
                        (PB, H, D))
                    nc.vector.scalar_tensor_tensor(
                        y[:, h * H:(h + 1) * H, :], xh[h][:], 1.0, rbc,
                        ALU.bypass, ALU.mult)
                # transpose 128x128 bf16 blocks on PE -> psum; fp8 conversion
                # happens in the PSUM->SBUF copy; store in DoubleRow layout.
                # Both k-halves land in one yT tile so the store is a single
                # DMA with 2KB contiguous per partition.
                yT = pool.tile([PB, KD, MB * PB], FP8, tag=f"yT{mi}")
                for k in range(KD):
                    pst = psum.tile([PB, MB * PB], BF16, tag=f"pst{mi}{k}")
                    for t in range(MB):
                        nc.tensor.transpose(
                            pst[:, t * PB:(t + 1) * PB],
                            y[:, t, k * PB:(k + 1) * PB], idt[:],
                        )
                    # both copies on DVE: ACT is phase1's busiest engine
                    # (squares + table loads), DVE has slack
                    nc.vector.tensor_copy(yT[:, k, :], pst[:])
                qin.dma_start(dst[:], yT[:])

        if loop_n:
            with tc.For_i(0, loop_n, 1):
                body()
        else:
            body()
    nc.compile()
    return nc


def _build_phase2(loop_n=0, hoist_loads=False):
    """Similarity slab + three-engine streamed masked sum of exp.

    Inputs (per core, column-rolled so own rows are columns [0, 1024)):
      urT, vrT [128, 2, 8192] fp8; eyp/eym [128, 128] fp8 (I, -MASK*I).
    Output: negsum [128, 8] f32; negsum[p, m] = sum_j exp((s_ij - 1)*C) over
      both matrices, j != i for uu, for local row m*128 + p.
    """
    nc = bacc.Bacc("TRN2", target_bir_lowering=False, debug=False)
    urT = nc.dram_tensor("urT", [PB, KD, B], FP8, kind="ExternalInput")
    vrT = nc.dram_tensor("vrT", [PB, KD, B], FP8, kind="ExternalInput")
    eyp = nc.dram_tensor("eyp", [PB, PB], FP8, kind="ExternalInput")
    negsum = nc.dram_tensor("negsum", [PB, MB], F32, kind="ExternalOutput")

    plan = _plan_exp_schedule()

    with tile.TileContext(nc) as tc, ExitStack() as ctx:
        consts = ctx.enter_context(tc.tile_pool(name="consts", bufs=1))
        statp = ctx.enter_context(tc.tile_pool(name="stat", bufs=2))
        big = ctx.enter_context(tc.tile_pool(name="big", bufs=2))
        esc = ctx.enter_context(tc.tile_pool(name="esc", bufs=3))
        eip = ctx.enter_context(tc.tile_pool(name="eip", bufs=6))
        psum = ctx.enter_context(
            tc.tile_pool(name="psum", bufs=2, space=bass.MemorySpace.PSUM)
        )

        eypt = consts.tile([PB, PB], FP8, tag="eypt")
        nc.sync.dma_start(eypt[:], eyp[:])

        biasc = consts.tile([PB, 1], F32, tag="biasc")
        nc.gpsimd.memset(biasc[:], -C)
        # trigger the exp ACT table load early (overlaps input DMA)
        actwarm = consts.tile([PB, 1], F32, tag="actwarm")
        nc.scalar.activation(actwarm[:], biasc[:], ACT.Exp,
                             bias=biasc[:], scale=C)

        def load_part():
            # input segments: [128, 2, 2048] fp8 tiles
            xT = {}

            def load(eng, nm, g, split_first=0):
                src = urT if nm == "u" else vrT
                t = big.tile([PB, KD, GROUP], FP8, tag=f"{nm}T{g}",
                             name=f"{nm}T{g}")
                lo = g * GROUP
                if split_first:
                    eng.dma_start(t[:, :, 0:split_first],
                                  src[:, :, lo:lo + split_first])
                else:
                    eng.dma_start(t[:], src[:, :, lo:lo + GROUP])
                xT[(nm, g)] = t

            # u-g0's first half holds every stationary block -> smallest
            # possible DMA first so statT staging starts ~3us earlier; v
            # groups in use order on the sync HWDGE ring, u groups on the
            # scalar HWDGE ring
            load(nc.sync, "u", 0, split_first=SH)
            load(nc.sync, "v", 0)
            nc.sync.dma_start(xT[("u", 0)][:, :, SH:GROUP],
                              urT[:, :, SH:GROUP])
            for g in range(1, NG):
                load(nc.sync, "v", g)
            for g in range(1, NG):
                load(nc.scalar, "u", g)

            # ldweights from a DMA-reloaded tile measured expensive on HW
            # (ring3 vs ring4); stage the stationary blocks in a tile that
            # is never a DMA target
            statT = big.tile([PB, KD, SH], FP8, tag="statT")
            nc.vector.tensor_copy(statT[:], xT[("u", 0)][:, :, 0:SH])
            return xT, statT

        def body(m_lo=0, m_hi=MB, emit_out=True, preloaded=None):
            xT, statT = preloaded if preloaded else load_part()

            # warm the PE (pstate ramp) while inputs stream in
            wps = psum.tile([PB, GROUP], F32, tag="ps")
            for _w in range(4):
                nc.tensor.matmul(wps[:, 0:PB], eypt[:], eypt[:],
                                 start=True, stop=True, skip_group_check=True)

            NSLOT = 2 * NG  # 8 tile slots per m, one grpsum column each
            # stat pool is double-buffered so, in the For_i timing builds,
            # iteration i+1's first accum_out into grpsum never waits on
            # iteration i's final negT row reduce (cross-iteration WAR)
            grpsum = statp.tile([PB, MB * NSLOT], F32, tag="grpsum")
            negT = statp.tile([PB, MB], F32, tag="negT")

            for m in range(m_lo, m_hi):
                lhsT = statT[:, :, m * PB:(m + 1) * PB]
                # uu group 0 (diag block) last, so its mask matmul doesn't
                # force a mid-stream stationary reload
                tiles = ([("v", g) for g in range(NG)]
                         + [("u", g) for g in (*range(1, NG), 0)])
                for slot, (nm, g) in enumerate(tiles):
                    ps = psum.tile([PB, GROUP], F32, tag="ps")
                    for q in range(NQ):
                        nc.tensor.matmul(
                            ps[:, q * CHUNK:(q + 1) * CHUNK],
                            lhsT,
                            xT[(nm, g)][:, :, q * CHUNK:(q + 1) * CHUNK],
                            start=True, stop=True,
                            perf_mode=DR, skip_group_check=True,
                        )
                    col = m * NSLOT + slot
                    eng = plan[(m, slot)]
                    if eng == "A":
                        escr = esc.tile([PB, GROUP], BF16, tag="escr")
                        nc.scalar.activation(
                            escr[:], ps[:], ACT.Exp,
                            bias=biasc[:], scale=C,
                            accum_out=grpsum[:, col:col + 1],
                        )
                    else:
                        # G: DVE-only bf16 Schraudolph: i16 = A16*C*ps + B,
                        # bitcast bf16 ~ exp(C*(ps-1)); then a 2-level
                        # pairwise bf16 tree-fold + reduce, all on DVE so
                        # nothing in DVE's in-order stream ever waits on
                        # another engine.
                        ei = eip.tile([PB, GROUP], BF16, tag="ei")
                        nc.vector.tensor_scalar(
                            ei[:].bitcast(I16), ps[:],
                            A16 * C, B16 - A16 * C, ALU.mult, ALU.add,
                        )
                        h = GROUP // 2
                        t1 = eip.tile([PB, h], BF16, tag="t1")
                        nc.vector.tensor_tensor(
                            t1[:], ei[:, 0:h], ei[:, h:GROUP], ALU.add)
                        q4 = GROUP // 4
                        t2 = eip.tile([PB, q4], BF16, tag="t2")
                        nc.vector.tensor_tensor(
                            t2[:], t1[:, 0:q4], t1[:, q4:h], ALU.add)
                        nc.vector.reduce_sum(
                            grpsum[:, col:col + 1], t2[:],
                            axis=mybir.AxisListType.X)
                    # interleave the previous m's tiny row reduce once its
                    # last column (slot 7's accum) has long been written
                    if slot == 4 and m > m_lo:
                        pm = m - 1
                        nc.vector.reduce_sum(
                            negT[:, pm:pm + 1],
                            grpsum[:, pm * NSLOT:(pm + 1) * NSLOT],
                            axis=mybir.AxisListType.X,
                        )

            pm = m_hi - 1
            nc.vector.reduce_sum(
                negT[:, pm:pm + 1],
                grpsum[:, pm * NSLOT:(pm + 1) * NSLOT],
                axis=mybir.AxisListType.X,
            )

            if emit_out:
                nc.sync.dma_start(negsum[:], negT[:])

        # single body measured best (149.9-154.1us); a two-half-loop split
        # measured 165us (duplicated input loads offset any fetch gains)
        if loop_n:
            pre = load_part() if hoist_loads else None
            with tc.For_i(0, loop_n, 1):
                body(preloaded=pre)
        else:
            body()
    nc.compile()
    return nc


def _get_programs():
    if "p1" not in _PROGRAMS:
        _PROGRAMS["p1"] = _build_phase1()
        _PROGRAMS["p2"] = _build_phase2()
    return _PROGRAMS["p1"], _PROGRAMS["p2"]


def _np_fp8():
    import ml_dtypes
    return ml_dtypes.float8_e4m3


def make_phase1_inputs(u, v):
    import ml_dtypes
    eye8 = np.eye(PB, dtype=ml_dtypes.bfloat16)
    return [
        {"us": u[c * SH:(c + 1) * SH], "vs": v[c * SH:(c + 1) * SH],
         "id8": eye8}
        for c in range(NCORES)
    ]


def make_phase2_inputs(unT, vnT):
    """Per-core phase-2 inputs from the 8 normalized transposed fp8 shards
    [128, 2, 1024], column-rolled so each core's own rows come first."""
    fp8 = _np_fp8()
    eyp = np.eye(PB, dtype=fp8)
    in2 = []
    for c in range(NCORES):
        in2.append({
            "urT": np.concatenate(unT[c:] + unT[:c], axis=2),
            "vrT": np.concatenate(vnT[c:] + vnT[:c], axis=2),
            "eyp": eyp,
        })
    return in2


def run_phases(u, v):
    """Returns (loss_scalar, phase1_results, phase2_results)."""
    u = np.ascontiguousarray(np.asarray(u, dtype=np.float32))
    v = np.ascontiguousarray(np.asarray(v, dtype=np.float32))
    assert u.shape == (B, D) and v.shape == (B, D)
    p1, p2 = _get_programs()
    cores = list(range(NCORES))

    in1 = make_phase1_inputs(u, v)
    r1 = run_bass_kernel_spmd(p1, in1, cores)
    unT = [r1.results[c]["unT"] for c in cores]
    vnT = [r1.results[c]["vnT"] for c in cores]

    in2 = make_phase2_inputs(unT, vnT)
    r2 = run_bass_kernel_spmd(p2, in2, cores)
    negs = np.stack(
        [np.asarray(r2.results[c]["negsum"], dtype=np.float64) for c in cores]
    )  # [8, 128, 8]; [c, p, m] -> urT column c*1024 + m*128 + p
    negsum = negs.transpose(0, 2, 1).reshape(B)  # column order

    # exact device fp8 unit-u rows, for the uu-diag correction
    uf8 = np.concatenate(
        [t.astype(np.float64).reshape(D, SH) for t in
         (x.reshape(PB * KD, SH) for x in unT)], axis=1)  # [256, 8192]
    d_uu = (uf8 * uf8).sum(axis=0)

    loss = _host_tail(u, v, negsum, d_uu)
    return np.float32(loss), r1, r2


# phase-1 layout: urT column j*128 + p within a shard holds shard row
# 8p + j (row r -> partition r//8, line r%8). negsum/d_uu arrive in
# column order; COLPERM maps column index -> global row index.
_j = np.arange(SH)
_COLPERM_LOCAL = 8 * (_j % PB) + (_j // PB)
COLPERM = (np.arange(NCORES)[:, None] * SH
           + _COLPERM_LOCAL[None, :]).reshape(B)


def _host_tail(u, v, negsum, d_uu):
    """loss_i = ln(negsum_i - diag corrections) + C - C*d_i, mean over rows.

    Both diagonals are left in on the device and removed here: the uv diag
    via the exact f32 cos, the uu diag from the device's own fp8 unit rows
    (phase-1 output bytes), so only the f32 summation-order mismatch vs the
    PE (~1e-6 relative) remains."""
    u64 = u.astype(np.float64)
    v64 = v.astype(np.float64)
    un = u64 / np.linalg.norm(u64, axis=1, keepdims=True)
    vn = v64 / np.linalg.norm(v64, axis=1, keepdims=True)
    d = np.einsum("ij,ij->i", un, vn)[COLPERM]  # into column order
    corr = np.exp((d - 1.0) * C) + np.exp((d_uu - 1.0) * C)
    loss = np.log(np.maximum(negsum - corr, 1e-300)) + C - C * d
    return loss.mean()


def kernel(u, v):
    out, _, _ = run_phases(u, v)
    return np.asarray(out, dtype=np.float32)


if __name__ == "__main__":
    rng = np.random.default_rng(0)
    u = rng.standard_normal((B, D), dtype=np.float32)
    v = rng.standard_normal((B, D), dtype=np.float32)
    print("loss:", kernel(u, v))



# revision 42
# speedup vs baseline: 1.0182x; 1.0182x over previous
"""DCL contrastive loss kernel for Trainium2 (8 NeuronCores, Bass/Tile).

Problem: u, v [8192, 256] f32.
  sim_uv = cos_sim(u, v) / T ; sim_uu = cos_sim(u, u) / T   (T = 0.07)
  loss = mean_i( -sim_uv[i,i] + logsumexp_j(off-diag of [sim_uv | sim_uu] row i) )

Strategy (data-parallel rows, per the sharding hint), v3:
  Phase 1 (SPMD, 8 cores): each core normalizes its 1024-row shard of u and v
    (ACT Square+accum row norms, DVE reciprocal), emits FP8 (e4m3) unit rows
    TRANSPOSED into the DoubleRow matmul layout [128, 2, 1024]. Input rows
    map r -> (partition r//8, line r%8) so every DMA line is >=4KB
    contiguous; the resulting column permutation (col j*128+p <-> shard row
    8p+j) is undone on the host (COLPERM). All DMAs ride the two HWDGE
    rings (sync + scalar) -- gpsimd SWDGE descriptor emission measured
    ~60us of pure overhead here.
  Host: concatenate the 8 shards column-ROLLED per core (own rows first) so
    every core's uu diagonal block sits at a static column offset.
  Phase 2 (SPMD, 8 cores): each core computes its [1024 x 8192] slab of both
    similarity matrices with fp8 DoubleRow matmuls (0.5 cyc/col, full K=256
    per instruction) into a 2-deep [128, 2048] PSUM ring, drained by TWO
    engines in parallel (HW-calibrated 44:20 tile split):
      - ACT "A" tiles: hardware exp activation, fused row-sum accum_out
        (1.66us/tile).
      - DVE "G" tiles: bf16 Schraudolph exp (i16 = A16*C*ps + B16, bitcast
        bf16 ~ exp(C(ps-1))), then a 2-level pairwise bf16 tree-fold and a
        reduce -- ALL on DVE (3.6us/tile), so nothing in DVE's in-order
        stream ever waits on another engine (cross-engine chains through
        GPSIMD measured 30-50us of head-of-line stalls and were removed).
    Both diagonals are left in on the device; the host subtracts their
    exact contributions (uv diag from the f64 cos, uu diag from the
    device's own fp8 unit rows, so the subtraction matches the device
    bytes bit-for-bit up to f32 summation order).
    Output: negsum [128, 8] f32 per core (row sums of exp((s-1)C) over
    both matrices, diags included), in urT column order.
  Host: loss_i = ln(negsum_i - diag corrections) + C - C*d_i with d_i the
    exact f64 diag cos(u_i, v_i) taken in column order; mean in f64.

The `loop_n` build parameter wraps the phase body in an on-device For_i loop
(used only for benchmarking; launch overhead cancels via the loop delta).
"""

import os
import sys

for _p in ("/opt/trn_rl_repo",):
    if _p not in sys.path:
        sys.path.insert(0, _p)

os.environ.setdefault("BASS_NEVER_TRACE", "1")

from contextlib import ExitStack

import numpy as np

import concourse.bass as bass
import concourse.tile as tile
from concourse import bacc, mybir
from concourse.bass_utils import run_bass_kernel_spmd

NCORES = 8
B, D = 8192, 256
SH = B // NCORES      # 1024 rows per core
PB = 128              # partition block
MB = SH // PB         # 8 row blocks per core
KD = D // PB          # 2 contraction halves
TEMP = 0.07
C = float(1.0 / TEMP)
SEG = 2048            # columns per DMA input tile
NSEG = B // SEG       # 4 input segments per matrix
GROUP = 2048          # columns per exp/accumulate group (4 PSUM banks)
NG = B // GROUP       # 4 groups per matrix
CHUNK = 512           # matmul chunk (1 PSUM bank)
NQ = GROUP // CHUNK
MASK = 6.0            # uu diag mask: s -> s - MASK => exp arg ~ -C*(1+MASK)

# Schraudolph: i32 = A_S*x + B_S, bitcast -> ~e^x  (zero-mean-error shift)
_LOG2E = 1.4426950408889634
A_S = float((1 << 23) * _LOG2E)
B_S = float(127.0 * (1 << 23) - 0.056446 * (1 << 23))
# bf16 variant: i16 = A16*x + B16, bitcast bf16 -> ~e^x
A16 = float((1 << 7) * _LOG2E)
B16 = float(127.0 * (1 << 7) - 0.056446 * (1 << 7))

F32 = mybir.dt.float32
BF16 = mybir.dt.bfloat16
FP8 = mybir.dt.float8e4
I32 = mybir.dt.int32
I16 = mybir.dt.int16
ALU = mybir.AluOpType
ACT = mybir.ActivationFunctionType
DR = mybir.MatmulPerfMode.DoubleRow

_PROGRAMS = {}

# --- static two-engine drain schedule for the 64 exp tiles ---------------


def _plan_exp_schedule():
    """Static balanced assignment of the 128 (m, slot) [128,1024] exp tiles:
    'A' = ACT exp with fused row-sum accumulator, 'G' = DVE Schraudolph feed
    consumed by a gpsimd per-m accumulator chain (sim-measured rates: ACT
    ~1164 ns, DVE feed ~1237 ns, GP add ~985 ns, per-m acc reduce ~1124 ns
    -> ACT/DVE/GP land near 82 us). Interleaved for the 4-deep PSUM ring."""
    # HW-calibrated rates: ACT exp 425 ns, DVE feed 647 ns, DVE bitcast
    # reduce ~2.1 us, gpsimd add ~2.6 us (serial chain). ACT-heavy is right:
    # 13 ACT + 3 gpsimd-chain tiles per m balances ACT/DVE/GP near 43 us.
    # Mixed G/A: with no SWDGE DMA emission left on the Pool queue (all
    # DMAs moved to the two HWDGE rings), the gpsimd chain is viable again.
    # Sim rates: ACT exp 2.07us, DVE feed 2.13us, GP add 4.16us per
    # [128,2048] tile. Alternating 3G/4G rows balance all three engines
    # near 75-83us total (ACT 74.5, DVE 76.6 w/ folds, GP 83).
    # The uu diag (slot 7) rides through exp unmasked on either engine
    # (arg ~0 -> i16 ~ 16249, safely positive in bf16 Schraudolph) and is
    # subtracted exactly on the host from the device's own fp8 bytes.
    # G tiles are fully self-contained on DVE (feed 2.26us from PSUM +
    # bf16 tree-fold 0.92us + reduce 0.59us = 3.77us), so no cross-engine
    # latency ever enters DVE's in-order stream. ACT tile = 2.08us.
    # 6:2 mix of 3G/2G rows balances ACT 87us / DVE 85us.
    # G spread 0/3/6 so the 2-deep PSUM ring always has an ACT tile and a
    # DVE tile adjacent (no same-reader serialization)
    # HW-calibrated (microbench 2026-08-10): ACT exp 1.657us/tile, DVE
    # G-chain 3.60us/tile, PE stream 1.39us/tile, mixed pattern paces at
    # ~1.67us/tile. Balance 44A:20G -> ACT 72.9us ~= DVE 72us.
    pat3 = ["G", "A", "A", "G", "A", "A", "G", "A"]
    pat2 = ["G", "A", "A", "A", "G", "A", "A", "A"]
    plan = {}
    for m in range(MB):
        pat = pat2 if m % 2 == 1 else pat3
        for slot in range(2 * NG):
            plan[(m, slot)] = pat[slot]
    return plan


def _build_phase1(loop_n=0):
    """Normalize shard rows, emit fp8 transposed DoubleRow layout:
    us, vs [1024, 256] f32 -> unT, vnT [128, 2, 1024] fp8."""
    nc = bacc.Bacc("TRN2", target_bir_lowering=False, debug=False)
    us = nc.dram_tensor("us", [SH, D], F32, kind="ExternalInput")
    vs = nc.dram_tensor("vs", [SH, D], F32, kind="ExternalInput")
    id8 = nc.dram_tensor("id8", [PB, PB], BF16, kind="ExternalInput")
    unT = nc.dram_tensor("unT", [PB, KD, SH], FP8, kind="ExternalOutput")
    vnT = nc.dram_tensor("vnT", [PB, KD, SH], FP8, kind="ExternalOutput")

    with tile.TileContext(nc) as tc, ExitStack() as ctx:
        pool = ctx.enter_context(tc.tile_pool(name="main", bufs=2))
        sp = ctx.enter_context(tc.tile_pool(name="small", bufs=4))
        consts = ctx.enter_context(tc.tile_pool(name="consts", bufs=1))
        psum = ctx.enter_context(
            tc.tile_pool(name="psum", bufs=2, space=bass.MemorySpace.PSUM)
        )
        idt = consts.tile([PB, PB], BF16, tag="idt")
        nc.sync.dma_start(idt[:], id8[:])

        def body():
            for mi, (src, dst) in enumerate(((us, unT), (vs, vnT))):
                # both queues are HWDGE rings (SP + ACT): gpsimd SWDGE
                # descriptor emission (~5-25us/DMA) was phase1's bottleneck
                qin = nc.sync if mi == 0 else nc.scalar
                ss = sp.tile([PB, MB], F32, tag=f"ss{mi}", name=f"ss{mi}")
                H = MB // 2
                xh = []
                # row r -> (p, t) = (r // MB, r % MB): each partition line
                # is 4KB contiguous (4x1KB rows) instead of 4 scattered 1KB
                # rows -- fewer, larger DMA descriptors. The resulting
                # column permutation (urT col j*128+p <-> shard row 8p+j)
                # is undone on the host (COLPERM).
                xsrc = src.rearrange("(p t) d -> p t d", p=PB)
                for h in range(2):
                    x = pool.tile([PB, H, D], F32, tag=f"x{mi}{h}",
                                  name=f"x{mi}{h}")
                    qin.dma_start(x[:], xsrc[:, h * H:(h + 1) * H, :])
                    xh.append(x)
                    sq = pool.tile([PB, H, D], F32, tag=f"sq{mi}{h}",
                                   name=f"sq{mi}{h}")
                    for t in range(H):
                        nc.scalar.activation(
                            sq[:, t, :], x[:, t, :], ACT.Square,
                            accum_out=ss[:, h * H + t:h * H + t + 1])
                # r = 1/sqrt(ss); fp8 output tolerates the raw reciprocal
                # (ACT.Rsqrt is rejected by bass for accuracy reasons)
                nrm = sp.tile([PB, MB], F32, tag=f"nrm{mi}")
                nc.scalar.activation(nrm[:], ss[:], ACT.Sqrt)
                rn = sp.tile([PB, MB], F32, tag=f"rn{mi}")
                nc.vector.reciprocal(rn[:], nrm[:])
                # y = x * r -> bf16, one fused broadcast multiply per half
                y = pool.tile([PB, MB, D], BF16, tag=f"y{mi}")
                for h in range(2):
                    rbc = rn[:, h * H:(h + 1) * H].unsqueeze(-1).broadcast_to(
                        (PB, H, D))
                    nc.vector.scalar_tensor_tensor(
                        y[:, h * H:(h + 1) * H, :], xh[h][:], 1.0, rbc,
                        ALU.bypass, ALU.mult)
                # transpose 128x128 bf16 blocks on PE -> psum; fp8 conversion
                # happens in the PSUM->SBUF copy; store in DoubleRow layout.
                # Both k-halves land in one yT tile so the store is a single
                # DMA with 2KB contiguous per partition.
                yT = pool.tile([PB, KD, MB * PB], FP8, tag=f"yT{mi}")
                for k in range(KD):
                    pst = psum.tile([PB, MB * PB], BF16, tag=f"pst{mi}{k}")
                    for t in range(MB):
                        nc.tensor.transpose(
                            pst[:, t * PB:(t + 1) * PB],
                            y[:, t, k * PB:(k + 1) * PB], idt[:],
                        )
                    # both copies on DVE: ACT is phase1's busiest engine
                    # (squares + table loads), DVE has slack
                    nc.vector.tensor_copy(yT[:, k, :], pst[:])
                qin.dma_start(dst[:], yT[:])

        if loop_n:
            with tc.For_i(0, loop_n, 1):
                body()
        else:
            body()
    nc.compile()
    return nc


def _build_phase2(loop_n=0, hoist_loads=False):
    """Similarity slab + three-engine streamed masked sum of exp.

    Inputs (per core, column-rolled so own rows are columns [0, 1024)):
      urT, vrT [128, 2, 8192] fp8; eyp/eym [128, 128] fp8 (I, -MASK*I).
    Output: negsum [128, 8] f32; negsum[p, m] = sum_j exp((s_ij - 1)*C) over
      both matrices, j != i for uu, for local row m*128 + p.
    """
    nc = bacc.Bacc("TRN2", target_bir_lowering=False, debug=False)
    urT = nc.dram_tensor("urT", [PB, KD, B], FP8, kind="ExternalInput")
    vrT = nc.dram_tensor("vrT", [PB, KD, B], FP8, kind="ExternalInput")
    eyp = nc.dram_tensor("eyp", [PB, PB], FP8, kind="ExternalInput")
    negsum = nc.dram_tensor("negsum", [PB, MB], F32, kind="ExternalOutput")

    plan = _plan_exp_schedule()

    with tile.TileContext(nc) as tc, ExitStack() as ctx:
        consts = ctx.enter_context(tc.tile_pool(name="consts", bufs=1))
        statp = ctx.enter_context(tc.tile_pool(name="stat", bufs=2))
        big = ctx.enter_context(tc.tile_pool(name="big", bufs=2))
        esc = ctx.enter_context(tc.tile_pool(name="esc", bufs=3))
        eip = ctx.enter_context(tc.tile_pool(name="eip", bufs=6))
        psum = ctx.enter_context(
            tc.tile_pool(name="psum", bufs=2, space=bass.MemorySpace.PSUM)
        )

        eypt = consts.tile([PB, PB], FP8, tag="eypt")
        nc.sync.dma_start(eypt[:], eyp[:])

        biasc = consts.tile([PB, 1], F32, tag="biasc")
        nc.gpsimd.memset(biasc[:], -C)
        # trigger the exp ACT table load early (overlaps input DMA)
        actwarm = consts.tile([PB, 1], F32, tag="actwarm")
        nc.scalar.activation(actwarm[:], biasc[:], ACT.Exp,
                             bias=biasc[:], scale=C)

        def load_part():
            # input segments: [128, 2, 2048] fp8 tiles
            xT = {}

            def load(eng, nm, g, split_first=0):
                src = urT if nm == "u" else vrT
                t = big.tile([PB, KD, GROUP], FP8, tag=f"{nm}T{g}",
                             name=f"{nm}T{g}")
                lo = g * GROUP
                if split_first:
                    eng.dma_start(t[:, :, 0:split_first],
                                  src[:, :, lo:lo + split_first])
                else:
                    eng.dma_start(t[:], src[:, :, lo:lo + GROUP])
                xT[(nm, g)] = t

            # u-g0's first half holds every stationary block -> smallest
            # possible DMA first so statT staging starts ~3us earlier; v
            # groups in use order on the sync HWDGE ring, u groups on the
            # scalar HWDGE ring
            load(nc.sync, "u", 0, split_first=SH)
            load(nc.sync, "v", 0)
            nc.sync.dma_start(xT[("u", 0)][:, :, SH:GROUP],
                              urT[:, :, SH:GROUP])
            for g in range(1, NG):
                load(nc.sync, "v", g)
            for g in range(1, NG):
                load(nc.scalar, "u", g)

            # ldweights from a DMA-reloaded tile measured expensive on HW
            # (ring3 vs ring4); stage the stationary blocks in a tile that
            # is never a DMA target
            statT = big.tile([PB, KD, SH], FP8, tag="statT")
            nc.vector.tensor_copy(statT[:], xT[("u", 0)][:, :, 0:SH])
            return xT, statT

        def body(m_lo=0, m_hi=MB, emit_out=True, preloaded=None):
            xT, statT = preloaded if preloaded else load_part()

            # warm the PE (pstate ramp) while inputs stream in
            wps = psum.tile([PB, GROUP], F32, tag="ps")
            for _w in range(4):
                nc.tensor.matmul(wps[:, 0:PB], eypt[:], eypt[:],
                                 start=True, stop=True, skip_group_check=True)

            NSLOT = 2 * NG  # 8 tile slots per m, one grpsum column each
            # stat pool is double-buffered so, in the For_i timing builds,
            # iteration i+1's first accum_out into grpsum never waits on
            # iteration i's final negT row reduce (cross-iteration WAR)
            grpsum = statp.tile([PB, MB * NSLOT], F32, tag="grpsum")
            negT = statp.tile([PB, MB], F32, tag="negT")

            for m in range(m_lo, m_hi):
                lhsT = statT[:, :, m * PB:(m + 1) * PB]
                # uu group 0 (diag block) last, so its mask matmul doesn't
                # force a mid-stream stationary reload
                tiles = ([("v", g) for g in range(NG)]
                         + [("u", g) for g in (*range(1, NG), 0)])
                for slot, (nm, g) in enumerate(tiles):
                    ps = psum.tile([PB, GROUP], F32, tag="ps")
                    for q in range(NQ):
                        nc.tensor.matmul(
                            ps[:, q * CHUNK:(q + 1) * CHUNK],
                            lhsT,
                            xT[(nm, g)][:, :, q * CHUNK:(q + 1) * CHUNK],
                            start=True, stop=True,
                            perf_mode=DR, skip_group_check=True,
                        )
                    col = m * NSLOT + slot
                    eng = plan[(m, slot)]
                    if eng == "A":
                        escr = esc.tile([PB, GROUP], BF16, tag="escr")
                        nc.scalar.activation(
                            escr[:], ps[:], ACT.Exp,
                            bias=biasc[:], scale=C,
                            accum_out=grpsum[:, col:col + 1],
                        )
                    else:
                        # G: DVE-only bf16 Schraudolph: i16 = A16*C*ps + B,
                        # bitcast bf16 ~ exp(C*(ps-1)); then a 2-level
                        # pairwise bf16 tree-fold + reduce, all on DVE so
                        # nothing in DVE's in-order stream ever waits on
                        # another engine.
                        ei = eip.tile([PB, GROUP], BF16, tag="ei")
                        nc.vector.tensor_scalar(
                            ei[:].bitcast(I16), ps[:],
                            A16 * C, B16 - A16 * C, ALU.mult, ALU.add,
                        )
                        h = GROUP // 2
                        t1 = eip.tile([PB, h], BF16, tag="t1")
                        nc.vector.tensor_tensor(
                            t1[:], ei[:, 0:h], ei[:, h:GROUP], ALU.add)
                        q4 = GROUP // 4
                        t2 = eip.tile([PB, q4], BF16, tag="t2")
                        nc.vector.tensor_tensor(
                            t2[:], t1[:, 0:q4], t1[:, q4:h], ALU.add)
                        nc.vector.reduce_sum(
                            grpsum[:, col:col + 1], t2[:],
                            axis=mybir.AxisListType.X)
                    # interleave the previous m's tiny row reduce once its
                    # last column (slot 7's accum) has long been written
                    if slot == 4 and m > m_lo:
                        pm = m - 1
                        nc.vector.reduce_sum(
                            negT[:, pm:pm + 1],
                            grpsum[:, pm * NSLOT:(pm + 1) * NSLOT],
                            axis=mybir.AxisListType.X,
                        )

            pm = m_hi - 1
            nc.vector.reduce_sum(
                negT[:, pm:pm + 1],
                grpsum[:, pm * NSLOT:(pm + 1) * NSLOT],
                axis=mybir.AxisListType.X,
            )

            if emit_out:
                nc.sync.dma_start(negsum[:], negT[:])

        # single body measured best (149.9-154.1us); a two-half-loop split
        # measured 165us (duplicated input loads offset any fetch gains)
        if loop_n:
            pre = load_part() if hoist_loads else None
            with tc.For_i(0, loop_n, 1):
                body(preloaded=pre)
        else:
            body()
    nc.compile()
    return nc


def _get_programs():
    if "p1" not in _PROGRAMS:
        _PROGRAMS["p1"] = _build_phase1()
        _PROGRAMS["p2"] = _build_phase2()
    return _PROGRAMS["p1"], _PROGRAMS["p2"]


def _np_fp8():
    import ml_dtypes
    return ml_dtypes.float8_e4m3


def make_phase1_inputs(u, v):
    import ml_dtypes
    eye8 = np.eye(PB, dtype=ml_dtypes.bfloat16)
    return [
        {"us": u[c * SH:(c + 1) * SH], "vs": v[c * SH:(c + 1) * SH],
         "id8": eye8}
        for c in range(NCORES)
    ]


def make_phase2_inputs(unT, vnT):
    """Per-core phase-2 inputs from the 8 normalized transposed fp8 shards
    [128, 2, 1024], column-rolled so each core's own rows come first."""
    fp8 = _np_fp8()
    eyp = np.eye(PB, dtype=fp8)
    in2 = []
    for c in range(NCORES):
        in2.append({
            "urT": np.concatenate(unT[c:] + unT[:c], axis=2),
            "vrT": np.concatenate(vnT[c:] + vnT[:c], axis=2),
            "eyp": eyp,
        })
    return in2


def run_phases(u, v):
    """Returns (loss_scalar, phase1_results, phase2_results)."""
    u = np.ascontiguousarray(np.asarray(u, dtype=np.float32))
    v = np.ascontiguousarray(np.asarray(v, dtype=np.float32))
    assert u.shape == (B, D) and v.shape == (B, D)
    p1, p2 = _get_programs()
    cores = list(range(NCORES))

    in1 = make_phase1_inputs(u, v)
    r1 = run_bass_kernel_spmd(p1, in1, cores)
    unT = [r1.results[c]["unT"] for c in cores]
    vnT = [r1.results[c]["vnT"] for c in cores]

    in2 = make_phase2_inputs(unT, vnT)
    r2 = run_bass_kernel_spmd(p2, in2, cores)
    negs = np.stack(
        [np.asarray(r2.results[c]["negsum"], dtype=np.float64) for c in cores]
    )  # [8, 128, 8]; [c, p, m] -> urT column c*1024 + m*128 + p
    negsum = negs.transpose(0, 2, 1).reshape(B)  # column order

    # exact device fp8 unit-u rows, for the uu-diag correction
    uf8 = np.concatenate(
        [t.astype(np.float64).reshape(D, SH) for t in
         (x.reshape(PB * KD, SH) for x in unT)], axis=1)  # [256, 8192]
    d_uu = (uf8 * uf8).sum(axis=0)

    loss = _host_tail(u, v, negsum, d_uu)
    return np.float32(loss), r1, r2


# phase-1 layout: urT column j*128 + p within a shard holds shard row
# 8p + j (row r -> partition r//8, line r%8). negsum/d_uu arrive in
# column order; COLPERM maps column index -> global row index.
_j = np.arange(SH)
_COLPERM_LOCAL = 8 * (_j % PB) + (_j // PB)
COLPERM = (np.arange(NCORES)[:, None] * SH
           + _COLPERM_LOCAL[None, :]).reshape(B)


def _host_tail(u, v, negsum, d_uu):
    """loss_i = ln(negsum_i - diag corrections) + C - C*d_i, mean over rows.

    Both diagonals are left in on the device and removed here: the uv diag
    via the exact f32 cos, the uu diag from the device's own fp8 unit rows
    (phase-1 output bytes), so only the f32 summation-order mismatch vs the
    PE (~1e-6 relative) remains."""
    u64 = u.astype(np.float64)
    v64 = v.astype(np.float64)
    un = u64 / np.linalg.norm(u64, axis=1, keepdims=True)
    vn = v64 / np.linalg.norm(v64, axis=1, keepdims=True)
    d = np.einsum("ij,ij->i", un, vn)[COLPERM]  # into column order
    corr = np.exp((d - 1.0) * C) + np.exp((d_uu - 1.0) * C)
    loss = np.log(np.maximum(negsum - corr, 1e-300)) + C - C * d
    return loss.mean()


def kernel(u, v):
    out, _, _ = run_phases(u, v)
    return np.asarray(out, dtype=np.float32)


if __name__ == "__main__":
    rng = np.random.default_rng(0)
    u = rng.standard_normal((B, D), dtype=np.float32)
    v = rng.standard_normal((B, D), dtype=np.float32)
    print("loss:", kernel(u, v))



# revision 43
# speedup vs baseline: 1.1056x; 1.0858x over previous
"""DCL contrastive loss kernel for Trainium2 (8 NeuronCores, Bass/Tile).

Problem: u, v [8192, 256] f32.
  sim_uv = cos_sim(u, v) / T ; sim_uu = cos_sim(u, u) / T   (T = 0.07)
  loss = mean_i( -sim_uv[i,i] + logsumexp_j(off-diag of [sim_uv | sim_uu] row i) )

Strategy (data-parallel rows, per the sharding hint), v3:
  Phase 1 (SPMD, 8 cores): each core normalizes its 1024-row shard of u and v
    (ACT Square+accum row norms, DVE reciprocal), emits FP8 (e4m3) unit rows
    TRANSPOSED into the DoubleRow matmul layout [128, 2, 1024]. Input rows
    map r -> (partition r//8, line r%8) so every DMA line is >=4KB
    contiguous; the resulting column permutation (col j*128+p <-> shard row
    8p+j) is undone on the host (COLPERM). All DMAs ride the two HWDGE
    rings (sync + scalar) -- gpsimd SWDGE descriptor emission measured
    ~60us of pure overhead here.
  Host: concatenate the 8 shards column-ROLLED per core (own rows first) so
    every core's uu diagonal block sits at a static column offset.
  Phase 2 (SPMD, 8 cores): each core computes its [1024 x 8192] slab of both
    similarity matrices with fp8 DoubleRow matmuls (0.5 cyc/col, full K=256
    per instruction) into a 2-deep [128, 2048] PSUM ring, drained by TWO
    engines in parallel (HW-calibrated 44:20 tile split):
      - ACT "A" tiles: hardware exp activation, fused row-sum accum_out
        (1.66us/tile).
      - DVE "G" tiles: bf16 Schraudolph exp (i16 = A16*C*ps + B16, bitcast
        bf16 ~ exp(C(ps-1))), then a 2-level pairwise bf16 tree-fold and a
        reduce -- ALL on DVE (3.6us/tile), so nothing in DVE's in-order
        stream ever waits on another engine (cross-engine chains through
        GPSIMD measured 30-50us of head-of-line stalls and were removed).
    Both diagonals are left in on the device; the host subtracts their
    exact contributions (uv diag from the f64 cos, uu diag from the
    device's own fp8 unit rows, so the subtraction matches the device
    bytes bit-for-bit up to f32 summation order).
    Output: negsum [128, 8] f32 per core (row sums of exp((s-1)C) over
    both matrices, diags included), in urT column order.
  Host: loss_i = ln(negsum_i - diag corrections) + C - C*d_i with d_i the
    exact f64 diag cos(u_i, v_i) taken in column order; mean in f64.

The `loop_n` build parameter wraps the phase body in an on-device For_i loop
(used only for benchmarking; launch overhead cancels via the loop delta).
"""

import os
import sys

for _p in ("/opt/trn_rl_repo",):
    if _p not in sys.path:
        sys.path.insert(0, _p)

os.environ.setdefault("BASS_NEVER_TRACE", "1")

from contextlib import ExitStack

import numpy as np

import concourse.bass as bass
import concourse.tile as tile
from concourse import bacc, mybir
from concourse.bass_utils import run_bass_kernel_spmd

NCORES = 8
B, D = 8192, 256
SH = B // NCORES      # 1024 rows per core
PB = 128              # partition block
MB = SH // PB         # 8 row blocks per core
KD = D // PB          # 2 contraction halves
TEMP = 0.07
C = float(1.0 / TEMP)
SEG = 2048            # columns per DMA input tile
NSEG = B // SEG       # 4 input segments per matrix
GROUP = 2048          # columns per exp/accumulate group (4 PSUM banks)
NG = B // GROUP       # 4 groups per matrix
CHUNK = 512           # matmul chunk (1 PSUM bank)
NQ = GROUP // CHUNK
MASK = 6.0            # uu diag mask: s -> s - MASK => exp arg ~ -C*(1+MASK)

# Schraudolph: i32 = A_S*x + B_S, bitcast -> ~e^x  (zero-mean-error shift)
_LOG2E = 1.4426950408889634
A_S = float((1 << 23) * _LOG2E)
B_S = float(127.0 * (1 << 23) - 0.056446 * (1 << 23))
# bf16 variant: i16 = A16*x + B16, bitcast bf16 -> ~e^x
A16 = float((1 << 7) * _LOG2E)
B16 = float(127.0 * (1 << 7) - 0.056446 * (1 << 7))

F32 = mybir.dt.float32
BF16 = mybir.dt.bfloat16
FP8 = mybir.dt.float8e4
I32 = mybir.dt.int32
I16 = mybir.dt.int16
ALU = mybir.AluOpType
ACT = mybir.ActivationFunctionType
DR = mybir.MatmulPerfMode.DoubleRow

_PROGRAMS = {}

# --- static two-engine drain schedule for the 64 exp tiles ---------------


def _plan_exp_schedule():
    """Static balanced assignment of the 128 (m, slot) [128,1024] exp tiles:
    'A' = ACT exp with fused row-sum accumulator, 'G' = DVE Schraudolph feed
    consumed by a gpsimd per-m accumulator chain (sim-measured rates: ACT
    ~1164 ns, DVE feed ~1237 ns, GP add ~985 ns, per-m acc reduce ~1124 ns
    -> ACT/DVE/GP land near 82 us). Interleaved for the 4-deep PSUM ring."""
    # HW-calibrated rates: ACT exp 425 ns, DVE feed 647 ns, DVE bitcast
    # reduce ~2.1 us, gpsimd add ~2.6 us (serial chain). ACT-heavy is right:
    # 13 ACT + 3 gpsimd-chain tiles per m balances ACT/DVE/GP near 43 us.
    # Mixed G/A: with no SWDGE DMA emission left on the Pool queue (all
    # DMAs moved to the two HWDGE rings), the gpsimd chain is viable again.
    # Sim rates: ACT exp 2.07us, DVE feed 2.13us, GP add 4.16us per
    # [128,2048] tile. Alternating 3G/4G rows balance all three engines
    # near 75-83us total (ACT 74.5, DVE 76.6 w/ folds, GP 83).
    # The uu diag (slot 7) rides through exp unmasked on either engine
    # (arg ~0 -> i16 ~ 16249, safely positive in bf16 Schraudolph) and is
    # subtracted exactly on the host from the device's own fp8 bytes.
    # G tiles are fully self-contained on DVE (feed 2.26us from PSUM +
    # bf16 tree-fold 0.92us + reduce 0.59us = 3.77us), so no cross-engine
    # latency ever enters DVE's in-order stream. ACT tile = 2.08us.
    # 6:2 mix of 3G/2G rows balances ACT 87us / DVE 85us.
    # G spread 0/3/6 so the 2-deep PSUM ring always has an ACT tile and a
    # DVE tile adjacent (no same-reader serialization)
    # HW-calibrated (microbench 2026-08-10): ACT exp 1.657us/tile, DVE
    # G-chain 3.60us/tile, PE stream 1.39us/tile, mixed pattern paces at
    # ~1.67us/tile. Balance 44A:20G -> ACT 72.9us ~= DVE 72us.
    pat3 = ["G", "A", "A", "G", "A", "A", "G", "A"]
    pat2 = ["G", "A", "A", "A", "G", "A", "A", "A"]
    plan = {}
    for m in range(MB):
        pat = pat2 if m % 2 == 1 else pat3
        for slot in range(2 * NG):
            plan[(m, slot)] = pat[slot]
    return plan


def _build_phase1(loop_n=0):
    """Normalize shard rows, emit fp8 transposed DoubleRow layout:
    us, vs [1024, 256] f32 -> unT, vnT [128, 2, 1024] fp8."""
    nc = bacc.Bacc("TRN2", target_bir_lowering=False, debug=False)
    us = nc.dram_tensor("us", [SH, D], F32, kind="ExternalInput")
    vs = nc.dram_tensor("vs", [SH, D], F32, kind="ExternalInput")
    id8 = nc.dram_tensor("id8", [PB, PB], BF16, kind="ExternalInput")
    unT = nc.dram_tensor("unT", [PB, KD, SH], FP8, kind="ExternalOutput")
    vnT = nc.dram_tensor("vnT", [PB, KD, SH], FP8, kind="ExternalOutput")

    with tile.TileContext(nc) as tc, ExitStack() as ctx:
        pool = ctx.enter_context(tc.tile_pool(name="main", bufs=2))
        sp = ctx.enter_context(tc.tile_pool(name="small", bufs=4))
        consts = ctx.enter_context(tc.tile_pool(name="consts", bufs=1))
        psum = ctx.enter_context(
            tc.tile_pool(name="psum", bufs=2, space=bass.MemorySpace.PSUM)
        )
        idt = consts.tile([PB, PB], BF16, tag="idt")
        nc.sync.dma_start(idt[:], id8[:])

        def body():
            for mi, (src, dst) in enumerate(((us, unT), (vs, vnT))):
                # both queues are HWDGE rings (SP + ACT): gpsimd SWDGE
                # descriptor emission (~5-25us/DMA) was phase1's bottleneck
                qin = nc.sync if mi == 0 else nc.scalar
                ss = sp.tile([PB, MB], F32, tag=f"ss{mi}", name=f"ss{mi}")
                H = MB // 2
                xh = []
                # row r -> (p, t) = (r // MB, r % MB): each partition line
                # is 4KB contiguous (4x1KB rows) instead of 4 scattered 1KB
                # rows -- fewer, larger DMA descriptors. The resulting
                # column permutation (urT col j*128+p <-> shard row 8p+j)
                # is undone on the host (COLPERM).
                xsrc = src.rearrange("(p t) d -> p t d", p=PB)
                for h in range(2):
                    x = pool.tile([PB, H, D], F32, tag=f"x{mi}{h}",
                                  name=f"x{mi}{h}")
                    qin.dma_start(x[:], xsrc[:, h * H:(h + 1) * H, :])
                    xh.append(x)
                    sq = pool.tile([PB, H, D], F32, tag=f"sq{mi}{h}",
                                   name=f"sq{mi}{h}")
                    for t in range(H):
                        nc.scalar.activation(
                            sq[:, t, :], x[:, t, :], ACT.Square,
                            accum_out=ss[:, h * H + t:h * H + t + 1])
                # r = 1/sqrt(ss); fp8 output tolerates the raw reciprocal
                # (ACT.Rsqrt is rejected by bass for accuracy reasons)
                nrm = sp.tile([PB, MB], F32, tag=f"nrm{mi}")
                nc.scalar.activation(nrm[:], ss[:], ACT.Sqrt)
                rn = sp.tile([PB, MB], F32, tag=f"rn{mi}")
                nc.vector.reciprocal(rn[:], nrm[:])
                # y = x * r -> bf16, one fused broadcast multiply per half
                y = pool.tile([PB, MB, D], BF16, tag=f"y{mi}")
                for h in range(2):
                    rbc = rn[:, h * H:(h + 1) * H].unsqueeze(-1).broadcast_to(
                        (PB, H, D))
                    nc.vector.scalar_tensor_tensor(
                        y[:, h * H:(h + 1) * H, :], xh[h][:], 1.0, rbc,
                        ALU.bypass, ALU.mult)
                # transpose 128x128 bf16 blocks on PE -> psum; fp8 conversion
                # happens in the PSUM->SBUF copy; store in DoubleRow layout.
                # Both k-halves land in one yT tile so the store is a single
                # DMA with 2KB contiguous per partition.
                yT = pool.tile([PB, KD, MB * PB], FP8, tag=f"yT{mi}")
                for k in range(KD):
                    pst = psum.tile([PB, MB * PB], BF16, tag=f"pst{mi}{k}")
                    for t in range(MB):
                        nc.tensor.transpose(
                            pst[:, t * PB:(t + 1) * PB],
                            y[:, t, k * PB:(k + 1) * PB], idt[:],
                        )
                    # both copies on DVE: ACT is phase1's busiest engine
                    # (squares + table loads), DVE has slack
                    nc.vector.tensor_copy(yT[:, k, :], pst[:])
                qin.dma_start(dst[:], yT[:])

        if loop_n:
            with tc.For_i(0, loop_n, 1):
                body()
        else:
            body()
    nc.compile()
    return nc


def _build_phase2(loop_n=0, hoist_loads=False):
    """Similarity slab + three-engine streamed masked sum of exp.

    Inputs (per core, column-rolled so own rows are columns [0, 1024)):
      urT, vrT [128, 2, 8192] fp8; eyp/eym [128, 128] fp8 (I, -MASK*I).
    Output: negsum [128, 8] f32; negsum[p, m] = sum_j exp((s_ij - 1)*C) over
      both matrices, j != i for uu, for local row m*128 + p.
    """
    nc = bacc.Bacc("TRN2", target_bir_lowering=False, debug=False)
    urT = nc.dram_tensor("urT", [PB, KD, B], FP8, kind="ExternalInput")
    vrT = nc.dram_tensor("vrT", [PB, KD, B], FP8, kind="ExternalInput")
    eyp = nc.dram_tensor("eyp", [PB, PB], FP8, kind="ExternalInput")
    negsum = nc.dram_tensor("negsum", [PB, MB], F32, kind="ExternalOutput")

    plan = _plan_exp_schedule()

    with tile.TileContext(nc) as tc, ExitStack() as ctx:
        consts = ctx.enter_context(tc.tile_pool(name="consts", bufs=1))
        statp = ctx.enter_context(tc.tile_pool(name="stat", bufs=2))
        big = ctx.enter_context(tc.tile_pool(name="big", bufs=2))
        esc = ctx.enter_context(tc.tile_pool(name="esc", bufs=3))
        eip = ctx.enter_context(tc.tile_pool(name="eip", bufs=6))
        psum = ctx.enter_context(
            tc.tile_pool(name="psum", bufs=2, space=bass.MemorySpace.PSUM)
        )

        eypt = consts.tile([PB, PB], FP8, tag="eypt")
        nc.sync.dma_start(eypt[:], eyp[:])

        biasc = consts.tile([PB, 1], F32, tag="biasc")
        nc.gpsimd.memset(biasc[:], -C)
        # trigger the exp ACT table load early (overlaps input DMA)
        actwarm = consts.tile([PB, 1], F32, tag="actwarm")
        nc.scalar.activation(actwarm[:], biasc[:], ACT.Exp,
                             bias=biasc[:], scale=C)

        def load_part():
            # input segments: [128, 2, 2048] fp8 tiles
            xT = {}

            def load(eng, nm, g, split_first=0):
                src = urT if nm == "u" else vrT
                t = big.tile([PB, KD, GROUP], FP8, tag=f"{nm}T{g}",
                             name=f"{nm}T{g}")
                lo = g * GROUP
                if split_first:
                    eng.dma_start(t[:, :, 0:split_first],
                                  src[:, :, lo:lo + split_first])
                else:
                    eng.dma_start(t[:], src[:, :, lo:lo + GROUP])
                xT[(nm, g)] = t

            # u-g0's first half holds every stationary block -> smallest
            # possible DMA first so statT staging starts ~3us earlier; v
            # groups in use order on the sync HWDGE ring, u groups on the
            # scalar HWDGE ring
            load(nc.sync, "u", 0, split_first=SH)
            load(nc.sync, "v", 0)
            nc.sync.dma_start(xT[("u", 0)][:, :, SH:GROUP],
                              urT[:, :, SH:GROUP])
            for g in range(1, NG):
                load(nc.sync, "v", g)
            for g in range(1, NG):
                load(nc.scalar, "u", g)

            # ldweights from a DMA-reloaded tile measured expensive on HW
            # (ring3 vs ring4); stage the stationary blocks in a tile that
            # is never a DMA target
            statT = big.tile([PB, KD, SH], FP8, tag="statT")
            nc.vector.tensor_copy(statT[:], xT[("u", 0)][:, :, 0:SH])
            return xT, statT

        def body(m_lo=0, m_hi=MB, emit_out=True, preloaded=None):
            xT, statT = preloaded if preloaded else load_part()

            # warm the PE (pstate ramp) while inputs stream in
            wps = psum.tile([PB, GROUP], F32, tag="ps")
            for _w in range(4):
                nc.tensor.matmul(wps[:, 0:PB], eypt[:], eypt[:],
                                 start=True, stop=True, skip_group_check=True)

            NSLOT = 2 * NG  # 8 tile slots per m, one grpsum column each
            # stat pool is double-buffered so, in the For_i timing builds,
            # iteration i+1's first accum_out into grpsum never waits on
            # iteration i's final negT row reduce (cross-iteration WAR)
            grpsum = statp.tile([PB, MB * NSLOT], F32, tag="grpsum")
            negT = statp.tile([PB, MB], F32, tag="negT")

            for m in range(m_lo, m_hi):
                lhsT = statT[:, :, m * PB:(m + 1) * PB]
                # uu group 0 (diag block) last, so its mask matmul doesn't
                # force a mid-stream stationary reload
                tiles = ([("v", g) for g in range(NG)]
                         + [("u", g) for g in (*range(1, NG), 0)])
                for slot, (nm, g) in enumerate(tiles):
                    ps = psum.tile([PB, GROUP], F32, tag="ps")
                    for q in range(NQ):
                        nc.tensor.matmul(
                            ps[:, q * CHUNK:(q + 1) * CHUNK],
                            lhsT,
                            xT[(nm, g)][:, :, q * CHUNK:(q + 1) * CHUNK],
                            start=True, stop=True,
                            perf_mode=DR, skip_group_check=True,
                        )
                    col = m * NSLOT + slot
                    eng = plan[(m, slot)]
                    if eng == "A":
                        escr = esc.tile([PB, GROUP], BF16, tag="escr")
                        nc.scalar.activation(
                            escr[:], ps[:], ACT.Exp,
                            bias=biasc[:], scale=C,
                            accum_out=grpsum[:, col:col + 1],
                        )
                    else:
                        # G: DVE-only bf16 Schraudolph: i16 = A16*C*ps + B,
                        # bitcast bf16 ~ exp(C*(ps-1)); then a 2-level
                        # pairwise bf16 tree-fold + reduce, all on DVE so
                        # nothing in DVE's in-order stream ever waits on
                        # another engine.
                        ei = eip.tile([PB, GROUP], BF16, tag="ei")
                        nc.vector.tensor_scalar(
                            ei[:].bitcast(I16), ps[:],
                            A16 * C, B16 - A16 * C, ALU.mult, ALU.add,
                        )
                        h = GROUP // 2
                        t1 = eip.tile([PB, h], BF16, tag="t1")
                        nc.vector.tensor_tensor(
                            t1[:], ei[:, 0:h], ei[:, h:GROUP], ALU.add)
                        q4 = GROUP // 4
                        t2 = eip.tile([PB, q4], BF16, tag="t2")
                        nc.vector.tensor_tensor(
                            t2[:], t1[:, 0:q4], t1[:, q4:h], ALU.add)
                        nc.vector.reduce_sum(
                            grpsum[:, col:col + 1], t2[:],
                            axis=mybir.AxisListType.X)
                    # interleave the previous m's tiny row reduce once its
                    # last column (slot 7's accum) has long been written
                    if slot == 4 and m > m_lo:
                        pm = m - 1
                        nc.vector.reduce_sum(
                            negT[:, pm:pm + 1],
                            grpsum[:, pm * NSLOT:(pm + 1) * NSLOT],
                            axis=mybir.AxisListType.X,
                        )

            pm = m_hi - 1
            nc.vector.reduce_sum(
                negT[:, pm:pm + 1],
                grpsum[:, pm * NSLOT:(pm + 1) * NSLOT],
                axis=mybir.AxisListType.X,
            )

            if emit_out:
                # scalar ring, NOT sync: this DMA waits on the iteration's
                # very tail (last negT reduce), and HWDGE rings are strict
                # FIFO -- on the sync ring it would gate the next
                # iteration's u0a/v0 loads (needed immediately), whereas
                # the scalar ring's u-group loads have ~10us of slack.
                nc.scalar.dma_start(negsum[:], negT[:])

        # single body measured best (149.9-154.1us); a two-half-loop split
        # measured 165us (duplicated input loads offset any fetch gains)
        if loop_n:
            pre = load_part() if hoist_loads else None
            with tc.For_i(0, loop_n, 1):
                body(preloaded=pre)
        else:
            body()
    nc.compile()
    return nc


def _get_programs():
    if "p1" not in _PROGRAMS:
        _PROGRAMS["p1"] = _build_phase1()
        _PROGRAMS["p2"] = _build_phase2()
    return _PROGRAMS["p1"], _PROGRAMS["p2"]


def _np_fp8():
    import ml_dtypes
    return ml_dtypes.float8_e4m3


def make_phase1_inputs(u, v):
    import ml_dtypes
    eye8 = np.eye(PB, dtype=ml_dtypes.bfloat16)
    return [
        {"us": u[c * SH:(c + 1) * SH], "vs": v[c * SH:(c + 1) * SH],
         "id8": eye8}
        for c in range(NCORES)
    ]


def make_phase2_inputs(unT, vnT):
    """Per-core phase-2 inputs from the 8 normalized transposed fp8 shards
    [128, 2, 1024], column-rolled so each core's own rows come first."""
    fp8 = _np_fp8()
    eyp = np.eye(PB, dtype=fp8)
    in2 = []
    for c in range(NCORES):
        in2.append({
            "urT": np.concatenate(unT[c:] + unT[:c], axis=2),
            "vrT": np.concatenate(vnT[c:] + vnT[:c], axis=2),
            "eyp": eyp,
        })
    return in2


def run_phases(u, v):
    """Returns (loss_scalar, phase1_results, phase2_results)."""
    u = np.ascontiguousarray(np.asarray(u, dtype=np.float32))
    v = np.ascontiguousarray(np.asarray(v, dtype=np.float32))
    assert u.shape == (B, D) and v.shape == (B, D)
    p1, p2 = _get_programs()
    cores = list(range(NCORES))

    in1 = make_phase1_inputs(u, v)
    r1 = run_bass_kernel_spmd(p1, in1, cores)
    unT = [r1.results[c]["unT"] for c in cores]
    vnT = [r1.results[c]["vnT"] for c in cores]

    in2 = make_phase2_inputs(unT, vnT)
    r2 = run_bass_kernel_spmd(p2, in2, cores)
    negs = np.stack(
        [np.asarray(r2.results[c]["negsum"], dtype=np.float64) for c in cores]
    )  # [8, 128, 8]; [c, p, m] -> urT column c*1024 + m*128 + p
    negsum = negs.transpose(0, 2, 1).reshape(B)  # column order

    # exact device fp8 unit-u rows, for the uu-diag correction
    uf8 = np.concatenate(
        [t.astype(np.float64).reshape(D, SH) for t in
         (x.reshape(PB * KD, SH) for x in unT)], axis=1)  # [256, 8192]
    d_uu = (uf8 * uf8).sum(axis=0)

    loss = _host_tail(u, v, negsum, d_uu)
    return np.float32(loss), r1, r2


# phase-1 layout: urT column j*128 + p within a shard holds shard row
# 8p + j (row r -> partition r//8, line r%8). negsum/d_uu arrive in
# column order; COLPERM maps column index -> global row index.
_j = np.arange(SH)
_COLPERM_LOCAL = 8 * (_j % PB) + (_j // PB)
COLPERM = (np.arange(NCORES)[:, None] * SH
           + _COLPERM_LOCAL[None, :]).reshape(B)


def _host_tail(u, v, negsum, d_uu):
    """loss_i = ln(negsum_i - diag corrections) + C - C*d_i, mean over rows.

    Both diagonals are left in on the device and removed here: the uv diag
    via the exact f32 cos, the uu diag from the device's own fp8 unit rows
    (phase-1 output bytes), so only the f32 summation-order mismatch vs the
    PE (~1e-6 relative) remains."""
    u64 = u.astype(np.float64)
    v64 = v.astype(np.float64)
    un = u64 / np.linalg.norm(u64, axis=1, keepdims=True)
    vn = v64 / np.linalg.norm(v64, axis=1, keepdims=True)
    d = np.einsum("ij,ij->i", un, vn)[COLPERM]  # into column order
    corr = np.exp((d - 1.0) * C) + np.exp((d_uu - 1.0) * C)
    loss = np.log(np.maximum(negsum - corr, 1e-300)) + C - C * d
    return loss.mean()


def kernel(u, v):
    out, _, _ = run_phases(u, v)
    return np.asarray(out, dtype=np.float32)


if __name__ == "__main__":
    rng = np.random.default_rng(0)
    u = rng.standard_normal((B, D), dtype=np.float32)
    v = rng.standard_normal((B, D), dtype=np.float32)
    print("loss:", kernel(u, v))

